# revision 50
# baseline (speedup 1.0000x reference)
"""Multi-head causal attention (bs=4, seq=2048, dm=1024, 16 heads) on 8 trn2 cores.

Sharding: core c = (batch b = c//2, head-group g = c%2). Each core computes
QKV projections for its batch restricted to its 8 heads, causal attention for
those heads, and a partial output projection (contracting its 512 z-columns
with the matching 512 rows of w_out). The host sums the two partials per
batch and transposes (each core returns out^T [dm, seq] in bf16).

bf16 end-to-end (rel err ~5.6e-3 vs the 2e-2 gate). x is transposed during
load by the DMA XBAR so the PE never transposes. All loads ride the single
SP HWDGE queue grouped by DMA type (same-type same-queue DMAs run FIFO
back-to-back; each type alternation costs a ~2us completion-chained link),
ordered so arrivals lead the PE's consumption; the ones constants are Pool
memsets. The whole kernel is one fused PE stream: QKV projection of quarter
qq+1 and the output-projection groups are interleaved INTO the attention
item stream (preferentially at diagonal items, whose exp->mask chain is
longest), so the exp (ACT) load spreads across the full span and the PE
never drains. Attention items run a depth-6 software pipeline: scores+exp
for item i+6 are emitted before the PV of item i. Softmax denominators come
from a ones column appended to V; the z accumulator is copied to SBUF at
stop (frees its PSUM bank), then reciprocal (DVE) -> partition_broadcast
(Pool) -> scale (DVE) produce normalized z^T; the normalization is emitted
three items late so the in-order DVE queue never blocks on a PV-stop wait.
Output leaves per (row-tile, seq-block) so the store DMAs overlap the last
attention segment.
"""
import sys
sys.path.insert(0, "/opt/trn_rl_repo")

from collections import defaultdict
from contextlib import ExitStack

import numpy as np

import concourse.tile as tile
from concourse import bacc, mybir
from concourse import bass_utils

F32 = mybir.dt.float32
BF16 = mybir.dt.bfloat16
AF = mybir.ActivationFunctionType

BS, SEQ, DM, H, DK = 4, 2048, 1024, 16, 64
HL = 8          # heads per core
NPAIR = 4       # head pairs per core
N_CORES = 8
NQB = SEQ // 512    # 4 q blocks of 512
NKC = SEQ // 128    # 16 k chunks of 128
NDC = DM // 128     # 8 dm chunks

_CACHE = {}


def _build_nc(phases="ABC", repeat=1):
    nc = bacc.Bacc("TRN2", target_bir_lowering=False, debug=False,
                   enable_asserts=True, num_devices=N_CORES)

    x_d = nc.dram_tensor("x_b", [SEQ, DM], BF16, kind="ExternalInput").ap()
    wqk_d = nc.dram_tensor("w_qk", [DM, 1024], BF16, kind="ExternalInput").ap()
    wv_d = nc.dram_tensor("w_v", [DM, 512], BF16, kind="ExternalInput").ap()
    bqk_d = nc.dram_tensor("b_qk", [1024], F32, kind="ExternalInput").ap()
    bv_d = nc.dram_tensor("b_v", [512], BF16, kind="ExternalInput").ap()
    wo_d = nc.dram_tensor("w_out", [512, DM], BF16, kind="ExternalInput").ap()
    out_d = nc.dram_tensor("outT", [DM, SEQ], BF16, kind="ExternalOutput").ap()

    def emit_once(tc, const, qkvp):
        # persistent activations (bf16)
        qkT = qkvp.tile([128, 8, SEQ], BF16, tag="qkT")  # 0-3: qT pairs, 4-7: kT
        v4 = qkvp.tile([128, NKC, HL, 65], BF16, tag="v4")

        with tc.tile_pool(name="wpool", bufs=1) as wpool, \
             tc.tile_pool(name="xt", bufs=1) as xt_pool, \
             tc.tile_pool(name="zt", bufs=1) as zt_pool, \
             tc.tile_pool(name="pt", bufs=12) as pt_pool, \
             tc.tile_pool(name="zsb", bufs=3) as zsb_pool, \
             tc.tile_pool(name="nrm", bufs=4) as nrm_pool, \
             tc.tile_pool(name="ostage", bufs=6) as ostage:

            wqk_sb = wpool.tile([128, NDC, 1024], BF16, tag="wqk")
            xT = xt_pool.tile([128, NDC, SEQ], BF16, tag="xt")
            # All loads on the single SP HWDGE queue, grouped by DMA type:
            # consecutive same-type same-queue DMAs run FIFO back-to-back
            # (waits elided); each type/queue alternation costs a ~2us
            # completion-chained link. Order: quarter-0 x transposes, then
            # every weight copy, then the remaining x transposes — arrivals
            # stay ahead of the PE's consumption throughout.
            for d in range(4):
                nc.sync.dma_start(xT[:, d, 0:512], x_d[0:512, d * 128:(d + 1) * 128],
                                  transpose=True)
            for d in range(4):
                nc.sync.dma_start(wqk_sb[:, d, :], wqk_d[d * 128:(d + 1) * 128, :])
            for d in range(4, NDC):
                nc.sync.dma_start(xT[:, d, 0:512], x_d[0:512, d * 128:(d + 1) * 128],
                                  transpose=True)
            bqk_sb = const.tile([128, 8], F32, tag="bqk")
            for d in range(4, NDC):
                nc.sync.dma_start(wqk_sb[:, d, :], wqk_d[d * 128:(d + 1) * 128, :])
            nc.sync.dma_start(bqk_sb[:], bqk_d.rearrange("(t p) -> p t", p=128))
            bv_row = const.tile([1, 512], BF16, tag="bv")
            nc.sync.dma_start(bv_row[:], bv_d[None, :])
            wv_sb = wpool.tile([128, NDC, 512], BF16, tag="wv")
            nc.sync.dma_start(wv_sb[:], wv_d.rearrange("(c p) n -> p c n", p=128))
            wo_sb = wpool.tile([128, 4, DM], BF16, tag="wo")
            nc.sync.dma_start(wo_sb[:], wo_d.rearrange("(c p) n -> p c n", p=128))
            for qq in range(1, 4):
                for d in range(NDC):
                    nc.sync.dma_start(
                        xT[:, d, qq * 512:(qq + 1) * 512],
                        x_d[qq * 512:(qq + 1) * 512, d * 128:(d + 1) * 128],
                        transpose=True)
            ones_row = const.tile([1, 128], BF16, tag="ones")
            nc.gpsimd.memset(ones_row[:], 1.0)
            nc.gpsimd.memset(v4[:, :, :, 64:65], 1.0)

            zT = zt_pool.tile([128, NPAIR, SEQ], BF16, tag="zT")
            fill0 = nc.gpsimd.to_reg(0.0)

            # ---- quarter-0 QK projection: d-outer over 8 live PSUM banks
            # (accumulation tracks the DMA arrivals chunk by chunk) ----
            with tc.tile_pool(name="ps0", bufs=4, space="PSUM") as ps0:
                # two passes of 4 t-tiles: pass-1 drains overlap pass-2
                # matmuls, and the freed banks hand over to the B pools
                # without waiting on a bunched drain tail
                for half in range(2):
                    ts = range(4 * half, 4 * half + 4)
                    ps_t = {t: ps0.tile([128, 512], F32, tag="qk0",
                                        name=f"qk0_{t}") for t in ts}
                    for d in range(NDC - 1):
                        for t in ts:
                            nc.tensor.matmul(ps_t[t][:],
                                             wqk_sb[:, d, t * 128:(t + 1) * 128],
                                             xT[:, d, 0:512],
                                             start=(d == 0), stop=False)
                    for t in ts:
                        nc.tensor.matmul(ps_t[t][:],
                                         wqk_sb[:, NDC - 1, t * 128:(t + 1) * 128],
                                         xT[:, NDC - 1, 0:512],
                                         start=False, stop=True)
                        nc.vector.tensor_scalar_add(
                            qkT[:, t, 0:512], ps_t[t][:], bqk_sb[:, t:t + 1])

            with tc.tile_pool(name="pla", bufs=1, space="PSUM") as pool_a, \
                 tc.tile_pool(name="psst", bufs=2, space="PSUM") as psst, \
                 tc.tile_pool(name="psz", bufs=3, space="PSUM") as psz:

                def qk_group(qq, t):
                    ps = pool_a.tile([128, 512], F32, tag="pa", name="psqk")
                    for d in range(NDC):
                        nc.tensor.matmul(ps[:], wqk_sb[:, d, t * 128:(t + 1) * 128],
                                         xT[:, d, qq * 512:(qq + 1) * 512],
                                         start=(d == 0), stop=(d == NDC - 1))
                    nc.vector.tensor_scalar_add(
                        qkT[:, t, qq * 512:(qq + 1) * 512], ps[:], bqk_sb[:, t:t + 1])

                def v_group(qq, s):
                    kc_glob = qq * 4 + s
                    ps = pool_a.tile([128, 512], F32, tag="pa", name="psv")
                    for d in range(NDC):
                        nc.tensor.matmul(
                            ps[:], xT[:, d, kc_glob * 128:(kc_glob + 1) * 128],
                            wv_sb[:, d, :], start=(d == 0), stop=False)
                    nc.tensor.matmul(ps[:], ones_row[:], bv_row[:],
                                     start=False, stop=True)
                    nc.vector.tensor_copy(
                        v4[:, kc_glob, :, 0:64],
                        ps[:].rearrange("p (h e) -> p h e", h=HL))

                def c_group(t, s, tail=False):
                    if tail:
                        # reuse the attention-score arena (same tag/shape)
                        po2 = psst.tile([128, 2, 512], F32, tag="st", name="po2")
                        po = po2[:, 0, :]
                    else:
                        po = pool_a.tile([128, 512], F32, tag="pa", name="po")
                    for j in range(NPAIR):
                        nc.tensor.matmul(po[:], wo_sb[:, j, t * 128:(t + 1) * 128],
                                         zT[:, j, s * 512:(s + 1) * 512],
                                         start=(j == 0), stop=(j == NPAIR - 1))
                    so = ostage.tile([128, 512], BF16, tag="so", name="so")
                    nc.vector.tensor_copy(so[:], po[:])
                    nc.sync.dma_start(
                        out_d[t * 128:(t + 1) * 128, s * 512:(s + 1) * 512],
                        so[:])

                for s in range(4):
                    v_group(0, s)

                # ---- fused attention + interleaved proj/out-proj groups ----
                items = []
                seg_bounds = {}
                for qb in range(NQB if "B" in phases else 0):
                    st = len(items)
                    for j in range(NPAIR):
                        nk = 4 * (qb + 1)
                        for kc in range(nk):
                            items.append((qb, j, kc, nk))
                    seg_bounds[qb] = (st, len(items))

                extras_map = defaultdict(list)

                def schedule(qb, thunks):
                    # Prefer placing extras at DIAGONAL items: their exp ->
                    # affine_select chain (~2.4us) is the longest pt latency,
                    # and the extra group's matmuls cover it before the PV.
                    # Cap at en-2: the next segment's first S (emitted at
                    # i == en-1) must come after every group it reads.
                    st, en = seg_bounds[qb]
                    n = en - st
                    diag = [i for i in range(st, st + n - 10)
                            if items[i][2] >= 4 * items[i][0]]
                    m = len(thunks)
                    for k, th in enumerate(thunks):
                        if m <= len(diag):
                            pos = diag[(k * len(diag)) // m]
                        elif k < len(diag):
                            pos = diag[k]
                        else:
                            pos = st + min(n - 11, ((k + 1) * n) // (m + 1))
                        extras_map[pos].append(th)

                if "B" in phases:
                    for qb in range(3):
                        thunks = [(lambda t=t, qq=qb + 1: qk_group(qq, t))
                                  for t in range(8)]
                        thunks += [(lambda s=s, qq=qb + 1: v_group(qq, s))
                                   for s in range(4)]
                        schedule(qb, thunks)
                    if "C" in phases:
                        schedule(3, [(lambda t=t, s=s: c_group(t, s))
                                     for s in range(3) for t in range(8)])
                else:
                    for qq in range(1, 4):
                        for t in range(8):
                            qk_group(qq, t)
                        for s in range(4):
                            v_group(qq, s)

                pts = {}
                zmap = {}
                pending_norms = []

                def emit_norm_direct(qb, j, zA, zB):
                    # tail variant: normalize straight from PSUM (no SBUF
                    # bounce) -- shortest chain from last PV to zT
                    bcs = []
                    for zh in (zA, zB):
                        rec = nrm_pool.tile([1, 512], F32, tag="rec", name="rec")
                        nc.vector.reciprocal(rec[:], zh[64:65, :])
                        bcs.append(nrm_pool.tile([64, 512], F32, tag="bc",
                                                 name="bc"))
                        nc.gpsimd.partition_broadcast(bcs[-1][:], rec[:])
                    for hidx, zh in ((0, zA), (1, zB)):
                        nc.vector.tensor_mul(
                            zT[64 * hidx:64 * hidx + 64, j,
                               qb * 512:(qb + 1) * 512],
                            zh[0:64, :], bcs[hidx][:])

                def emit_norm(qb, j, zA, zB):
                    # copy to SBUF at once (frees the PSUM bank), then
                    # normalize: z / denom (denom = partition 64). Emitted
                    # a couple of items AFTER the unit's last PV so the
                    # in-order DVE queue never blocks on the PV-stop wait.
                    zss, bcs = [], []
                    for zh in (zA, zB):
                        zs = zsb_pool.tile([65, 512], F32, tag="zs", name="zs")
                        nc.vector.tensor_copy(zs[:], zh[:])
                        rec = nrm_pool.tile([1, 512], F32, tag="rec", name="rec")
                        nc.vector.reciprocal(rec[:], zs[64:65, :])
                        zss.append(zs)
                        bcs.append(nrm_pool.tile([64, 512], F32, tag="bc",
                                                 name="bc"))
                        nc.gpsimd.partition_broadcast(bcs[-1][:], rec[:])
                    for hidx in (0, 1):
                        nc.vector.tensor_mul(
                            zT[64 * hidx:64 * hidx + 64, j,
                               qb * 512:(qb + 1) * 512],
                            zss[hidx][0:64, :], bcs[hidx][:])

                def do_S(i):
                    qb, j, kc, nk = items[i]
                    qoff = max(0, kc * 128 - qb * 512)  # diag narrowing
                    w = 512 - qoff
                    stAB = psst.tile([128, 2, 512], F32, tag="st", name="stAB")
                    nc.tensor.matmul(
                        stAB[:, 0, qoff:],
                        qkT[0:64, 4 + j, kc * 128:(kc + 1) * 128],
                        qkT[0:64, j, qb * 512 + qoff:(qb + 1) * 512],
                        start=True, stop=True)
                    nc.tensor.matmul(
                        stAB[:, 1, qoff:],
                        qkT[64:128, 4 + j, kc * 128:(kc + 1) * 128],
                        qkT[64:128, j, qb * 512 + qoff:(qb + 1) * 512],
                        start=True, stop=True)
                    ptAB = pt_pool.tile([128, 2, 512], BF16, tag="pt", name="ptAB")
                    nc.scalar.activation(ptAB[:, :, qoff:], stAB[:, :, qoff:],
                                         AF.Exp, scale=0.125)
                    if kc >= 4 * qb:  # diagonal block: causal mask
                        nc.gpsimd.affine_select(
                            out=ptAB[:, :, qoff:], in_=ptAB[:, :, qoff:],
                            compare_op=mybir.AluOpType.is_ge, fill=fill0,
                            base=0, pattern=[[0, 2], [1, w]],
                            channel_multiplier=-1)
                    pts[i] = (ptAB, qoff)

                def do_PV(i):
                    qb, j, kc, nk = items[i]
                    if kc == 0:
                        zmap[(qb, j)] = (
                            psz.tile([65, 512], F32, tag="z", name="zA"),
                            psz.tile([65, 512], F32, tag="z", name="zB"))
                    zA, zB = zmap[(qb, j)]
                    ptAB, qoff = pts.pop(i)
                    nc.tensor.matmul(zA[:, qoff:], v4[:, kc, 2 * j, :],
                                     ptAB[:, 0, qoff:],
                                     start=(kc == 0), stop=(kc == nk - 1))
                    nc.tensor.matmul(zB[:, qoff:], v4[:, kc, 2 * j + 1, :],
                                     ptAB[:, 1, qoff:],
                                     start=(kc == 0), stop=(kc == nk - 1))
                    if kc == nk - 1:
                        if i >= len(items) - 2:
                            emit_norm_direct(qb, j, zA, zB)
                        else:
                            pending_norms.append((i + 3, (qb, j, zA, zB)))

                if items:
                    for k in range(min(10, len(items))):
                        do_S(k)
                    for i in range(len(items)):
                        if i + 10 < len(items):
                            do_S(i + 10)
                        for th in extras_map.get(i, ()):
                            th()
                        do_PV(i)
                        while pending_norms and pending_norms[0][0] <= i:
                            emit_norm(*pending_norms.pop(0)[1])
                    while pending_norms:
                        emit_norm(*pending_norms.pop(0)[1])

                # ---- out-projection tail: last seq block + output DMAs ----
                if "C" in phases:
                    for t in range(8):
                        c_group(t, 3, tail=True)

    with tile.TileContext(nc) as tc, ExitStack() as top:
        const = top.enter_context(tc.tile_pool(name="const", bufs=1))
        qkvp = top.enter_context(tc.tile_pool(name="qkvp", bufs=1))
        for _rep in range(repeat):
            emit_once(tc, const, qkvp)

    nc.compile()
    return nc


def get_nc(phases="ABC", repeat=1):
    key = (phases, repeat)
    if key not in _CACHE:
        _CACHE[key] = _build_nc(phases, repeat)
    return _CACHE[key]


def make_in_maps(x, w_qkv, b_qkv, w_out):
    import ml_dtypes
    bf16 = ml_dtypes.bfloat16
    x = np.asarray(x, dtype=np.float32)
    w_qkv = np.asarray(w_qkv, dtype=np.float32)
    b_qkv = np.asarray(b_qkv, dtype=np.float32)
    w_out = np.asarray(w_out, dtype=np.float32)
    in_maps = []
    for c in range(N_CORES):
        b, g = divmod(c, 2)
        cs = slice(512 * g, 512 * (g + 1))
        w_qk = np.ascontiguousarray(
            np.concatenate([w_qkv[:, cs],
                            w_qkv[:, 1024 + 512 * g:1024 + 512 * (g + 1)]],
                           axis=1)).astype(bf16)
        w_v = np.ascontiguousarray(
            w_qkv[:, 2048 + 512 * g:2048 + 512 * (g + 1)]).astype(bf16)
        b_qk = np.ascontiguousarray(
            np.concatenate([b_qkv[cs], b_qkv[1024 + 512 * g:1024 + 512 * (g + 1)]]))
        b_v = np.ascontiguousarray(
            b_qkv[2048 + 512 * g:2048 + 512 * (g + 1)]).astype(bf16)
        w_o = np.ascontiguousarray(w_out[512 * g:512 * (g + 1), :]).astype(bf16)
        in_maps.append({
            "x_b": np.ascontiguousarray(x[b]).astype(bf16),
            "w_qk": w_qk, "w_v": w_v, "b_qk": b_qk,
            "b_v": b_v, "w_out": w_o,
        })
    return in_maps


def gather_output(results, b_out):
    b_out = np.asarray(b_out, dtype=np.float32)
    outs = []
    for b in range(BS):
        pT = (results[2 * b]["outT"].astype(np.float32)
              + results[2 * b + 1]["outT"].astype(np.float32))  # [dm, seq]
        outs.append(pT.T + b_out[None, :])
    return np.stack(outs).astype(np.float32)


def kernel(x, w_qkv, b_qkv, w_out, b_out):
    nc = get_nc()
    in_maps = make_in_maps(x, w_qkv, b_qkv, w_out)
    res = bass_utils.run_bass_kernel_spmd(nc, in_maps,
                                          core_ids=list(range(N_CORES)))
    return gather_output(res.results, b_out)
